# revision 64
# baseline (speedup 1.0000x reference)
# Trainium2 Bass kernel for nn_DecoderBlock (masked self-attn + cross-attn +
# LFFN decoder block with "linear" softmax attention over the head dim).
#
# Sharding: data-parallel over batch - 16 batch elems / 8 cores = 2 per core.
#
# Implementation notes (per core):
#  - All large matmuls run in fp8 e4m3 with DoubleRow perf mode (two 128-deep
#    contraction chunks per instruction, 0.5 PE cycles/row).  Weights are
#    quantized host-side with a x64 power-of-2 scale; activation tensors get
#    power-of-2 quant scales chosen so fp8 values sit ~1-sigma near 1.0.  All
#    scales are folded into existing evacuation ops (exact powers of two).
#  - The residual add is folded into the PSUM accumulation group of the output
#    matmul via a scaled-identity stationary (psum += RS * nat), so layernorm
#    reads psum directly and no separate residual op exists.
#  - Layernorm rstd uses exp(-0.5*ln(var+eps)) in the attention phases (the
#    {ln, exp, identity} table) and a 2-step Newton rsqrt on the vector engine
#    in the LFFN phase, where silu is computed as x*(1+tanh(x/2))/2 (the
#    {exp, tanh, identity} table) - a post-compile pass re-segments the
#    activation-table loads so the whole kernel needs only two table loads.
#  - softmax(Q) right-operands are produced by PE transposes (tensor engine
#    is_transpose matmuls) into PSUM, not DMA transposes; the transposed fp8
#    x for the next phase's projections is rebuilt the same way from the
#    layernorm outputs.
#  - Bulk loads are issued from the gpsimd/Pool engine (SWDGE path) and
#    stores/dependent loads from the sync engine, which keeps the globally
#    exclusive HWDGE device nearly idle and every compute SEQ free.
#  - Natural-layout activations round-trip phase-to-phase through DRAM in
#    bf16.  Work is emitted stage-interleaved with all evacuation/normalize
#    ops statically load-balanced across the DVE / Act / Pool engines.
import numpy as np
import ml_dtypes

import concourse.bacc as bacc
import concourse.mybir as mybir
import concourse.tile as tile
from concourse.bass_utils import run_bass_kernel_spmd

H, D, DQ, BNK, HID = 8, 1024, 128, 512, 1024
B, S_T, S_M = 16, 1024, 2048
TAU = DQ ** 0.25
EPS = 1e-5
NEG = -200.0
N_CORES = 8
BPC = B // N_CORES

WS = 64.0          # weight quant scale (all weights are ~0.02-sigma)
QV = 256.0         # V' = V/rowsum(expK) quant scale
QB = 32.0          # Bm quant scale
QG = 8.0           # g1 (LFFN second bottleneck) quant scale
RS1 = QB * WS      # attention output psum scale (2048)
RS2 = QG * WS      # LFFN output psum scale (512)

f32 = mybir.dt.float32
bf16 = mybir.dt.bfloat16
f8 = mybir.dt.float8e4
AF = mybir.ActivationFunctionType
ALU = mybir.AluOpType
DR = mybir.MatmulPerfMode.DoubleRow
X_AX = mybir.AxisListType.X
bfd = ml_dtypes.bfloat16
f8d = ml_dtypes.float8_e4m3


def _ln_psum(nc, sb, acc0, acc1, eps_t, rs, dst_dram, out_dt, gb, gbi, tag,
             newton=False):
    """LN over a [128, 1024] row-tile held as two psum halves scaled by rs.

    Computes (x - mean(x)) * rsqrt(var(x) + EPS) with x = psum/rs, entirely
    from the scaled psum stats:  rstd_p = exp(-0.5*ln(var_p + rs^2*EPS))
    equals rstd/rs, so activation(psum, scale=rstd_p, bias=-mean_p*rstd_p)
    is exactly the normalized tile.  Returns the SBUF tile (dtype out_dt).
    """
    st6 = sb.tile([128, 2, 6], f32, tag="ln_st6", bufs=2, name="st6")
    nc.vector.bn_stats(st6[:, 0, :], acc0)
    nc.vector.bn_stats(st6[:, 1, :], acc1)
    mv = sb.tile([128, 2], f32, tag="ln_mv", bufs=2, name="mv")
    nc.vector.bn_aggr(mv[:], st6[:])
    if newton:
        # rstd/rs = rsqrt(var_p + rs^2 eps) via 2 Newton steps from x0=1/rs
        # (valid because the LN input is a unit-variance residual stream:
        #  var is within ~10% of 1, so convergence is ~1e-5 after 2 steps)
        vv = sb.tile([128, 1], f32, tag="ln_vv", bufs=2, name="vv")
        nc.vector.tensor_scalar(out=vv[:], in0=mv[:, 1:2],
                                scalar1=rs * rs * EPS, scalar2=None,
                                op0=ALU.add)
        u1 = sb.tile([128, 1], f32, tag="ln_u1", bufs=2, name="u1")
        nc.vector.tensor_scalar(out=u1[:], in0=vv[:], scalar1=1.0 / (rs * rs),
                                scalar2=None, op0=ALU.mult)
        w1 = sb.tile([128, 1], f32, tag="ln_w1", bufs=2, name="w1")
        nc.vector.tensor_scalar(out=w1[:], in0=u1[:], scalar1=-0.5, scalar2=1.5,
                                op0=ALU.mult, op1=ALU.add)
        x1 = sb.tile([128, 1], f32, tag="ln_x1", bufs=2, name="x1")
        nc.vector.tensor_scalar(out=x1[:], in0=w1[:], scalar1=1.0 / rs,
                                scalar2=None, op0=ALU.mult)
        u2 = sb.tile([128, 1], f32, tag="ln_u2", bufs=2, name="u2")
        nc.vector.tensor_tensor(out=u2[:], in0=x1[:], in1=x1[:], op=ALU.mult)
        nc.vector.tensor_tensor(out=u2[:], in0=u2[:], in1=vv[:], op=ALU.mult)
        nc.vector.tensor_scalar(out=u2[:], in0=u2[:], scalar1=-0.5, scalar2=1.5,
                                op0=ALU.mult, op1=ALU.add)
        rstd = sb.tile([128, 1], f32, tag="ln_rstd", bufs=2, name="rstd")
        nc.vector.tensor_tensor(out=rstd[:], in0=x1[:], in1=u2[:], op=ALU.mult)
    else:
        lnv = sb.tile([128, 1], f32, tag="ln_lnv", bufs=2, name="lnv")
        nc.scalar.activation(lnv[:], mv[:, 1:2], AF.Ln, bias=eps_t[:])
        rstd = sb.tile([128, 1], f32, tag="ln_rstd", bufs=2, name="rstd")
        nc.scalar.activation(rstd[:], lnv[:], AF.Exp, scale=-0.5)
    cneg = sb.tile([128, 1], f32, tag="ln_cneg", bufs=2, name="cneg")
    nc.vector.scalar_tensor_tensor(
        out=cneg[:], in0=mv[:, 0:1], scalar=-1.0, in1=rstd[:],
        op0=ALU.mult, op1=ALU.mult)
    yt = sb.tile([128, D], out_dt, tag=tag, bufs=2, name="yt")
    nc.scalar.activation(yt[:, 0:512], acc0, AF.Identity, scale=rstd[:], bias=cneg[:])
    nc.scalar.activation(yt[:, 512:1024], acc1, AF.Identity, scale=rstd[:], bias=cneg[:])
    if gb is not None:
        nc.vector.tensor_tensor(out=yt[:], in0=yt[:], in1=gb[gbi][:], op=ALU.mult)
        nc.vector.tensor_tensor(out=yt[:], in0=yt[:], in1=gb[gbi + 1][:], op=ALU.add)
    nc.sync.dma_start(dst_dram, yt[:])
    return yt


def _attn_phase(nc, tc, C, pools, masked, n_kv, xT, kv_dram, wqkv_dram, wo_dram,
                nat_dram, ynext_dram, xT_next, gb, gbi, xT_dram=None):
    """One attention phase for both batch elems.

    xT:      per-b SBUF tiles [128, 8192] f8 = x transposed (lhsT source)
    kv_dram: None (self-attn: K/V from xT) or per-b DRAM [128, 16384] f8
    nat_dram(b) -> DRAM AP [128, 8, 1024] bf16 natural-layout residual input
    ynext_dram(b, hb) -> DRAM AP [128, 1024] bf16 store target
    xT_next: per-b SBUF tiles [128, 8192] f8 to fill with this phase's
             transposed fp8 output (or None after the last attn phase)

    Emission is stage-interleaved (A h0, A h1, A2/B h0, A2/B h1, C h0, C h1)
    per batch elem so the in-order engine queues always have runnable work.
    """
    id1, idr = C["id1"], C["id2048"]
    sb, ps = pools

    ld_xT = xT is None
    if ld_xT:  # phase 1: transposed x comes from DRAM
        xT = [sb.tile([128, 8192], f8, tag=f"xT1_{b}", name=f"xT1_{b}")
              for b in range(BPC)]
        nc.gpsimd.dma_start(xT[0][:, 0:4096], xT_dram[0][:, 0:4096])
        nc.gpsimd.dma_start(xT[0][:, 4096:8192], xT_dram[0][:, 4096:8192])
    wqs = []
    for hg in range(2):
        wq = sb.tile([128, 12288], f8, tag="wqkv", bufs=2, name="wq")
        base = 12288 * hg
        for q in (1, 2, 0):  # K, V first: stage A starts sooner
            nc.gpsimd.dma_start(
                wq[:, 4096 * q:4096 * (q + 1)],
                wqkv_dram[:, base + 4096 * q:base + 4096 * (q + 1)])
        wqs.append(wq[:].rearrange("p (q k i c) -> p q k i c", q=3, k=4, i=2))
        if ld_xT and hg == 0:
            nc.gpsimd.dma_start(xT[1][:], xT_dram[1])
    wo = sb.tile([128, 8192], f8, tag="wo", bufs=2, name="wo")
    nc.gpsimd.dma_start(wo[:], wo_dram[:])
    wov = wo[:].rearrange("p (j i n) -> p j i n", j=4, i=2)

    def stage_a(kvv, hg):
        """K/V projections + fold softmax(K) denominator into V'."""
        wv6 = wqs[hg]
        ek = sb.tile([128, 8192], f8, tag="expk", bufs=2, name="ek")
        ev = sb.tile([128, 8192], f8, tag="expv", bufs=2, name="ev")
        ekm = ek[:, 0:n_kv * 512].rearrange("p (m c) -> p m c", m=n_kv)
        ekh = ek[:, 0:n_kv * 512].rearrange("p (m h q) -> p m h q", m=n_kv, h=4)
        evh = ev[:, 0:n_kv * 512].rearrange("p (m h q) -> p m h q", m=n_kv, h=4)
        ekg = ek[:, 0:n_kv * 512].rearrange("p (g two h q) -> p g two h q",
                                            g=n_kv // 2, two=2, h=4, q=128)
        evg = ev[:, 0:n_kv * 512].rearrange("p (g two h q) -> p g two h q",
                                            g=n_kv // 2, two=2, h=4, q=128)
        for smp in range(n_kv // 2):
            kps = ps.tile([128, 1024], f32, tag="kv", bufs=2, name="kps")
            vps = ps.tile([128, 1024], f32, tag="kv", bufs=2, name="vps")
            for i in range(2):
                sm = 2 * smp + i
                for kp in range(4):
                    lhsT = kvv[:, 2 * kp:2 * kp + 2, 128 * sm:128 * (sm + 1)]
                    nc.tensor.matmul(kps[:, 512 * i:512 * (i + 1)], lhsT,
                                     wv6[:, 1, kp],
                                     start=(kp == 0), stop=(kp == 3), perf_mode=DR)
                    nc.tensor.matmul(vps[:, 512 * i:512 * (i + 1)], lhsT,
                                     wv6[:, 2, kp],
                                     start=(kp == 0), stop=(kp == 3), perf_mode=DR)
            nc.scalar.activation(ekm[:, 2 * smp:2 * smp + 2, :], kps[:], AF.Exp,
                                 scale=1.0 / (WS * TAU))
            krs = sb.tile([128, 8], f32, tag="krs", bufs=2, name="krs")
            kv2 = krs[:].rearrange("p (two h) -> p two h", two=2, h=4)
            nc.vector.tensor_reduce(out=kv2, in_=ekg[:, smp], axis=X_AX,
                                    op=ALU.add)
            krr = sb.tile([128, 8], f32, tag="krr", bufs=2, name="krr")
            nc.vector.reciprocal(krr[:], krs[:])
            nc.vector.scalar_tensor_tensor(
                out=evg[:, smp],
                in0=vps[:].rearrange("p (two h q) -> p two h q", two=2, h=4),
                scalar=QV / WS,
                in1=krr[:].rearrange("p (two h) -> p two h", two=2, h=4)
                    .unsqueeze(3).broadcast_to([128, 2, 4, 128]),
                op0=ALU.mult, op1=ALU.mult)
        return ekh, evh

    def stage_a2(ekh, evh):
        """A[d,e] = softK^T V' per head (psum = QV*A); evac to bf16."""
        aps = ps.tile([128, 512], f32, tag="bq", bufs=3, name="aps")
        for hi in range(4):
            for smp in range(n_kv // 2):
                nc.tensor.matmul(
                    aps[:, 128 * hi:128 * (hi + 1)],
                    ekh[:, 2 * smp:2 * smp + 2, hi],
                    evh[:, 2 * smp:2 * smp + 2, hi],
                    start=(smp == 0), stop=(smp == n_kv // 2 - 1), perf_mode=DR)
        asb = sb.tile([128, 512], bf16, tag="asb", bufs=2, name="asb")
        nc.scalar.activation(asb[:], aps[:], AF.Identity, scale=1.0 / QV)
        return asb

    def stage_b(xv, hg):
        """Q proj + softmax over head dim + PE transpose -> softQ^T."""
        wv6 = wqs[hg]
        sqT = sb.tile([128, 4096], bf16, tag="sqT", bufs=2, name="sqT")
        sqTv = sqT[:].rearrange("p (h s) -> p h s", h=4)
        for stp in range(4):
            eq = sb.tile([128, 1024], bf16, tag="eq", bufs=2, name="eq")
            for i in range(2):
                st = 2 * stp + i
                qps = ps.tile([128, 512], f32, tag="bq", bufs=3, name="qps")
                for kp in range(4):
                    nc.tensor.matmul(
                        qps[:],
                        xv[:, 2 * kp:2 * kp + 2, 128 * st:128 * (st + 1)],
                        wv6[:, 0, kp], start=(kp == 0), stop=(kp == 3),
                        perf_mode=DR)
                if masked and st == 0:
                    nc.vector.tensor_tensor(out=qps[:], in0=qps[:],
                                            in1=C["mask"][:], op=ALU.add)
                nc.scalar.activation(eq[:, 512 * i:512 * (i + 1)], qps[:],
                                     AF.Exp, scale=1.0 / (WS * TAU))
            eqg = eq[:].rearrange("p (two h q) -> p two h q", two=2, h=4, q=128)
            qrs = sb.tile([128, 8], bf16, tag="qrs", bufs=2, name="qrs")
            with nc.allow_low_precision(reason="softmax denom, 4e-3 ok"):
                nc.vector.tensor_reduce(
                    out=qrs[:].rearrange("p (two h) -> p two h", two=2, h=4),
                    in_=eqg, axis=X_AX, op=ALU.add)
            qrr = sb.tile([128, 8], f32, tag="qrr", bufs=2, name="qrr")
            nc.vector.reciprocal(qrr[:], qrs[:])
            sq = sb.tile([128, 1024], bf16, tag="sq", bufs=2, name="sq")
            sqg = sq[:].rearrange("p (two h q) -> p two h q", two=2, h=4, q=128)
            nc.gpsimd.tensor_tensor(
                out=sqg, in0=eqg,
                in1=qrr[:].rearrange("p (two h) -> p two h", two=2, h=4)
                    .unsqueeze(3).broadcast_to([128, 2, 4, 128]), op=ALU.mult)
            tp = ps.tile([128, 1024], bf16, tag="tp", bufs=1, name="tp")
            for i in range(2):
                for hi in range(4):
                    nc.tensor.transpose(
                        tp[:, 512 * i + 128 * hi:512 * i + 128 * (hi + 1)],
                        sqg[:, i, hi, :], id1[:])
            nc.scalar.activation(
                sqTv[:, :, 256 * stp:256 * (stp + 1)]
                    .rearrange("p h (two s) -> p two h s", two=2, s=128),
                tp[:].rearrange("p (two h s) -> p two h s", two=2, h=4, s=128),
                AF.Identity)
        return sqTv

    def stage_c(b, hg, asb, sqTv, natv):
        """Bm, Wo + fused residual, LN, store; rebuild transposed fp8 x."""
        yts = []
        for hi in range(4):
            hb = 4 * hg + hi
            # bms is stored j-major ([j, r] with r contiguous) so the
            # DoubleRow Wo stationary slices have a contiguous inner dim
            bms = sb.tile([128, 1024], f8, tag="bms", bufs=2, name="bms")
            bmj = bms[:].rearrange("p (j r) -> p j r", j=8)
            bmrj = bms[:].rearrange("p (j r) -> p r j", j=8)
            for nh in range(2):
                bmt = ps.tile([128, 512], f32, tag="bq", bufs=3, name="bmt")
                nc.tensor.matmul(bmt[:], asb[:, 128 * hi:128 * (hi + 1)],
                                 sqTv[:, hi, 512 * nh:512 * (nh + 1)])
                dst = bmrj[:, 64 * nh:64 * (nh + 1), :]
                if nh == 0:
                    nc.scalar.activation(dst, bmt[:], AF.Identity, scale=QB)
                else:
                    nc.vector.tensor_scalar_mul(out=dst, in0=bmt[:], scalar1=QB)
            accs = []
            for nh in range(2):
                opsn = ps.tile([128, 512], f32, tag="bq", bufs=3, name="opsn")
                for jp in range(4):
                    nc.tensor.matmul(
                        opsn[:], bmj[:, 2 * jp:2 * jp + 2, :],
                        wov[:, jp, :, 512 * nh:512 * (nh + 1)],
                        start=(jp == 0), stop=False, perf_mode=DR,
                        skip_group_check=True)
                nc.tensor.matmul(
                    opsn[:], idr[:], natv[:, hb, 512 * nh:512 * (nh + 1)],
                    start=False, stop=True, skip_group_check=True)
                accs.append(opsn)
            yt = _ln_psum(nc, sb, accs[0][:], accs[1][:], C["eps1"], RS1,
                          ynext_dram(b, hb), bf16, gb, gbi, tag="yt")
            yts.append(yt)
        if xT_next is not None:
            xnv = xT_next[b][:].rearrange("p (k s) -> p k s", k=8)
            for hi in range(4):
                hb = 4 * hg + hi
                tp = ps.tile([128, 1024], bf16, tag="tp", bufs=1, name="tpx")
                for kc in range(8):
                    nc.tensor.transpose(tp[:, 128 * kc:128 * (kc + 1)],
                                        yts[hi][:, 128 * kc:128 * (kc + 1)],
                                        id1[:])
                nc.scalar.activation(
                    xnv[:, :, 128 * hb:128 * (hb + 1)],
                    tp[:].rearrange("p (k s) -> p k s", k=8), AF.Identity)

    for b in range(BPC):
        if kv_dram is None:
            kvt = xT[b]
        else:
            kvt = sb.tile([128, 16384], f8, tag="mem8", bufs=1, name="kvt")
            nc.gpsimd.dma_start(kvt[:], kv_dram[b])
        kvv = kvt[:].rearrange("p (k s) -> p k s", k=8)
        xv = xT[b][:].rearrange("p (k s) -> p k s", k=8)
        nat = sb.tile([128, 8192], bf16, tag="nat", bufs=1, name="nat")
        nc.sync.dma_start(nat[:], nat_dram(b))
        natv = nat[:].rearrange("p (t d) -> p t d", t=8)

        kv0 = stage_a(kvv, 0)
        sqT0 = stage_b(xv, 0)
        kv1 = stage_a(kvv, 1)
        asb0 = stage_a2(*kv0)
        sqT1 = stage_b(xv, 1)
        asb1 = stage_a2(*kv1)
        stage_c(b, 0, asb0, sqT0, natv)
        stage_c(b, 1, asb1, sqT1, natv)


def _lffn_phase(nc, tc, C, lw, xT, nat_dram, out_dram, gb, gbi):
    idr = C["id512"]
    with tc.tile_pool(name="ffn_sb", bufs=1) as sb:
        e1v = lw[:, 0:4096].rearrange("p (k t i c) -> p k t i c", k=4, t=4, i=2)
        d1v = lw[:, 4096:8192].rearrange("p (k t i c) -> p k t i c", k=2, t=8, i=2)
        e2v = lw[:, 8192:12288].rearrange("p (k t i c) -> p k t i c", k=4, t=4, i=2)
        d2v = lw[:, 12288:16384].rearrange("p (k i n) -> p k i n", k=2, i=2)

        with tc.tile_pool(name="ffn_ps", bufs=1, space="PSUM") as ps:
            xvs, natvs, h1vs, swvs, g1vs = [], [], [], [], []
            for b in range(BPC):
                xvs.append(xT[b][:].rearrange("p (k s) -> p k s", k=8))
                nat = sb.tile([128, 8192], bf16, tag="nat3", bufs=2, name="nat")
                nc.sync.dma_start(nat[:], nat_dram(b))
                natvs.append(nat[:].rearrange("p (t d) -> p t d", t=8))
                h1 = sb.tile([128, 4096], f8, tag="h1", bufs=2, name="h1")
                h1vs.append(h1[:].rearrange("p (t s) -> p t s", t=4))
                sw = sb.tile([128, 8192], f8, tag="sw", bufs=2, name="sw")
                swvs.append(sw[:].rearrange("p (t s) -> p t s", t=8))
                g1 = sb.tile([128, 4096], f8, tag="g1", bufs=2, name="g1")
                g1vs.append(g1[:].rearrange("p (t s) -> p t s", t=4))

            # stage-major, batch-interleaved emission keeps the in-order
            # engine queues saturated across the LFFN dependency chain.
            # All stages run at [128, 1024] psum-pair width (2 banks).
            for b in range(BPC):
                for t in range(4):
                    acc = ps.tile([128, 1024], f32, tag="fa", bufs=4, name="acc")
                    for nh in range(2):
                        for kp in range(4):
                            nc.tensor.matmul(
                                acc[:, 512 * nh:512 * (nh + 1)], e1v[:, kp, t],
                                xvs[b][:, 2 * kp:2 * kp + 2, 512 * nh:512 * (nh + 1)],
                                start=(kp == 0), stop=(kp == 3), perf_mode=DR)
                    nc.vector.tensor_scalar_mul(out=h1vs[b][:, t, :],
                                                in0=acc[:], scalar1=1.0 / WS)
            for b in range(BPC):
                for t in range(8):
                    acc = ps.tile([128, 1024], f32, tag="fa", bufs=4, name="acc")
                    for nh in range(2):
                        for kp in range(2):
                            nc.tensor.matmul(
                                acc[:, 512 * nh:512 * (nh + 1)], d1v[:, kp, t],
                                h1vs[b][:, 2 * kp:2 * kp + 2, 512 * nh:512 * (nh + 1)],
                                start=(kp == 0), stop=(kp == 1), perf_mode=DR)
                    # silu via tanh: sw = 64h2*(1+tanh(h2/2)) = 128*silu(h2)
                    th = sb.tile([128, 1024], f32, tag="th", bufs=2, name="th")
                    nc.scalar.activation(th[:], acc[:], AF.Tanh,
                                         scale=0.5 / WS)
                    nc.vector.scalar_tensor_tensor(
                        out=swvs[b][:, t, :], in0=th[:], scalar=1.0,
                        in1=acc[:], op0=ALU.add, op1=ALU.mult)
            for b in range(BPC):
                for t in range(4):
                    acc = ps.tile([128, 1024], f32, tag="fa", bufs=4, name="acc")
                    for nh in range(2):
                        for kp in range(4):
                            nc.tensor.matmul(
                                acc[:, 512 * nh:512 * (nh + 1)], e2v[:, kp, t],
                                swvs[b][:, 2 * kp:2 * kp + 2, 512 * nh:512 * (nh + 1)],
                                start=(kp == 0), stop=(kp == 3), perf_mode=DR)
                    nc.scalar.activation(g1vs[b][:, t, :], acc[:], AF.Identity,
                                         scale=QG / (2.0 * WS * WS))
            for b in range(BPC):
                for st in range(8):
                    acc = ps.tile([128, 1024], f32, tag="fa", bufs=4, name="acc")
                    for nh in range(2):
                        for kp in range(2):
                            nc.tensor.matmul(
                                acc[:, 512 * nh:512 * (nh + 1)],
                                g1vs[b][:, 2 * kp:2 * kp + 2, 128 * st:128 * (st + 1)],
                                d2v[:, kp, :, 512 * nh:512 * (nh + 1)],
                                start=(kp == 0), stop=False, perf_mode=DR,
                                skip_group_check=True)
                        nc.tensor.matmul(
                            acc[:, 512 * nh:512 * (nh + 1)], idr[:],
                            natvs[b][:, st, 512 * nh:512 * (nh + 1)],
                            start=False, stop=True, skip_group_check=True)
                    _ln_psum(nc, sb, acc[:, 0:512], acc[:, 512:1024], C["eps2"],
                             RS2, out_dram[b, 128 * st:128 * (st + 1), :], f32,
                             gb, gbi, tag="yt3", newton=True)


def _build(affine: bool):
    nc = bacc.Bacc("TRN2", target_bir_lowering=False, debug=False,
                   enable_asserts=True, num_devices=N_CORES)

    def din(name, shape, dt):
        return nc.dram_tensor(name, list(shape), dt, kind="ExternalInput").ap()

    y0b = din("y0b", [BPC, 128, 8192], bf16)      # natural y, [p, hb, d] packed
    y0T8 = din("y0T8", [BPC, 128, 8192], f8)      # transposed y, [p, k, s]
    memT8 = din("memT8", [BPC, 128, 16384], f8)   # transposed mem, [p, k, sm]
    wqkv1 = din("wqkv1", [128, 24576], f8)
    wqkv2 = din("wqkv2", [128, 24576], f8)
    wo1 = din("wo1", [128, 8192], f8)
    wo2 = din("wo2", [128, 8192], f8)
    lffn = din("lffn", [128, 16384], f8)
    mask64 = din("mask64", [128, 512], f32)
    identp = din("identp", [128, 384], bf16)      # [I | RS1*I | RS2*I]
    if affine:
        grep = din("grep", [6, 128, D], f32)

    out = nc.dram_tensor("out", [BPC, S_T, D], f32, kind="ExternalOutput").ap()

    with tile.TileContext(nc) as tc:
        with tc.tile_pool(name="dram", bufs=1, space="DRAM") as dpool:
            y1d = dpool.tile([BPC, 8, 128, D], bf16)
            y2d = dpool.tile([BPC, 8, 128, D], bf16)

            with tc.tile_pool(name="glob", bufs=1) as gp:
                idp = gp.tile([128, 384], bf16, tag="idp", name="idp")
                nc.gpsimd.dma_start(idp[:], identp[:])
                maskt = gp.tile([128, 512], f32, tag="maskt", name="maskt")
                nc.gpsimd.dma_start(maskt[:], mask64[:])
                eps1 = gp.tile([128, 1], f32, tag="eps1", name="eps1")
                nc.vector.memset(eps1[:], RS1 * RS1 * EPS)
                eps2 = gp.tile([128, 1], f32, tag="eps2", name="eps2")
                nc.vector.memset(eps2[:], RS2 * RS2 * EPS)
                one_t = gp.tile([128, 1], f32, tag="one_t", name="one_t")
                nc.vector.memset(one_t[:], 1.0)
                C = {"id1": idp[:, 0:128], "id2048": idp[:, 128:256],
                     "id512": idp[:, 256:384], "mask": maskt, "eps1": eps1,
                     "eps2": eps2, "one": one_t}
                gb = None
                if affine:
                    gb = [gp.tile([128, D], f32, tag=f"gb{i}", name=f"gb{i}")
                          for i in range(6)]
                    for i in range(6):
                        nc.gpsimd.dma_start(gb[i][:], grep[i])

                xT2 = [gp.tile([128, 8192], f8, tag=f"xT2_{b}", name=f"xT2_{b}")
                       for b in range(BPC)]
                xT3 = [gp.tile([128, 8192], f8, tag=f"xT3_{b}", name=f"xT3_{b}")
                       for b in range(BPC)]
                lw = gp.tile([128, 16384], f8, tag="lw", name="lw")

                with tc.tile_pool(name="attn_sb", bufs=1) as asb, \
                        tc.tile_pool(name="attn_ps", bufs=1, space="PSUM") as aps:
                    pools = (asb, aps)
                    _attn_phase(
                        nc, tc, C, pools, masked=True, n_kv=8, xT=None,
                        xT_dram=y0T8, kv_dram=None, wqkv_dram=wqkv1, wo_dram=wo1,
                        nat_dram=lambda b: y0b[b],
                        ynext_dram=lambda b, hb: y1d[b, hb],
                        xT_next=xT2, gb=gb, gbi=0)
                    _attn_phase(
                        nc, tc, C, pools, masked=False, n_kv=16, xT=xT2,
                        kv_dram=memT8, wqkv_dram=wqkv2, wo_dram=wo2,
                        nat_dram=lambda b: y1d[b].transpose([1, 0, 2]),
                        ynext_dram=lambda b, hb: y2d[b, hb],
                        xT_next=xT3, gb=gb, gbi=2)
                    nc.gpsimd.dma_start(lw[:], lffn[:])
                _lffn_phase(
                    nc, tc, C, lw=lw[:], xT=xT3,
                    nat_dram=lambda b: y2d[b].transpose([1, 0, 2]),
                    out_dram=out, gb=gb, gbi=4)

    nc.compile()
    _dedup_act_table_loads(nc)
    return nc


def _dedup_act_table_loads(nc):
    """Replace the compiler's greedy per-func ATL placement (which thrashes
    between exp_and_others and natural_log for an Exp/Ln mix) with a minimal
    greedy segmentation: at each point where the running func set no longer
    fits one table, keep one ATL pointing at the table that covers the
    longest upcoming stretch of activations."""
    from concourse.hw_specs import get_activation_tables
    tables = get_activation_tables(nc.m.arch)
    names = list(tables)

    # ordered walk: activation funcs + ATL positions
    items = []  # (block, inst, kind)
    for b in nc.main_func.blocks:
        for i in list(b.instructions):
            if isinstance(i, mybir.InstLoadActFuncSet):
                items.append((b, i, "atl"))
            elif isinstance(i, mybir.InstActivation):
                items.append((b, i, "act"))
    funcs = [i.func for _, i, k in items if k == "act"]
    if not funcs:
        return
    # greedy: longest-prefix table cover
    segs = []  # (start_idx_in_funcs, table_idx)
    pos = 0
    while pos < len(funcs):
        best_len, best_tbl = 0, None
        for idx, n in enumerate(names):
            t = tables[n]
            ln = 0
            while pos + ln < len(funcs) and funcs[pos + ln] in t:
                ln += 1
            if ln > best_len:
                best_len, best_tbl = ln, idx
        assert best_tbl is not None, f"no table for {funcs[pos]}"
        segs.append((pos, best_tbl))
        pos += best_len
    seg_starts = {s: t for s, t in segs}
    # rewrite: keep an ATL before each segment-start activation, drop others
    act_i = 0
    pending_tbl = None
    for b, inst, kind in items:
        if kind == "atl":
            b.instructions.remove(inst)
            continue
        if act_i in seg_starts:
            tbl = seg_starts[act_i]
            atl = mybir.InstLoadActFuncSet(
                name=nc.get_next_instruction_name(), ins=[], outs=[],
                act_func_set_id=tbl)
            atl.engine = inst.engine
            nc.register_instruction(atl)
            idx = b.instructions.index(inst)
            b.instructions.insert(idx, atl)
        act_i += 1


_CACHE = {}


def _prep_host(inputs):
    """Pack/transpose/quantize weights + activations for the kernel layout."""
    g = {k: np.asarray(v) for k, v in inputs.items()}
    affine = not (
        np.all(g["g1"] == 1) and np.all(g["g2"] == 1) and np.all(g["g3"] == 1)
        and np.all(g["b1"] == 0) and np.all(g["b2"] == 0) and np.all(g["b3"] == 0))

    def wqkv_pack(q, k, v):
        # [3][H, D, DQ] -> [128, (q3 g2 kp4 i2 (hi4 dq128))] * WS in fp8
        w = np.stack([q, k, v]).astype(np.float64) * WS      # [3, 8, 1024, 128]
        w = w.reshape(3, 2, 4, 4, 2, 128, 128)               # q g hi kp i p dq
        w = w.transpose(5, 1, 0, 3, 4, 2, 6)                 # p g q kp i hi dq
        return np.ascontiguousarray(w.reshape(128, 24576)).astype(f8d)

    def wo_pack(wo):
        w = np.ascontiguousarray(wo.T).astype(np.float64) * WS   # WoT [1024,1024]
        w = w.reshape(4, 2, 128, 1024).transpose(2, 0, 1, 3)     # p jp i n
        return np.ascontiguousarray(w.reshape(128, 8192)).astype(f8d)

    def enc_pack(wt, kp, t):
        # wt [in_dim, out_dim] already transposed torch weight; -> [p kp t i c]
        w = wt.astype(np.float64) * WS
        w = w.reshape(kp, 2, 128, t, 128).transpose(2, 0, 3, 1, 4)
        return np.ascontiguousarray(w.reshape(128, kp * t * 2 * 128))

    host = {}
    host["wqkv1"] = wqkv_pack(g["Wq1"], g["Wk1"], g["Wv1"])
    host["wqkv2"] = wqkv_pack(g["Wq2"], g["Wk2"], g["Wv2"])
    host["wo1"] = wo_pack(g["Wo1"])
    host["wo2"] = wo_pack(g["Wo2"])
    e1 = enc_pack(np.ascontiguousarray(g["E1"].T), 4, 4)
    d1 = enc_pack(np.ascontiguousarray(g["D1"].T), 2, 8)
    e2 = enc_pack(np.ascontiguousarray(g["E2"].T), 4, 4)
    d2 = (np.ascontiguousarray(g["D2"].T).astype(np.float64) * WS) \
        .reshape(2, 2, 128, 1024).transpose(2, 0, 1, 3).reshape(128, 4096)
    host["lffn"] = np.ascontiguousarray(
        np.concatenate([e1, d1, e2, d2], axis=1)).astype(f8d)
    mask = np.where(np.arange(DQ)[None, :] <= np.arange(128)[:, None],
                    0.0, NEG * WS).astype(np.float32)
    host["mask64"] = np.ascontiguousarray(np.tile(mask, (1, 4)))
    idp = np.zeros((128, 384), np.float32)
    idp[:, 0:128] = np.eye(128)
    idp[:, 128:256] = np.eye(128) * RS1
    idp[:, 256:384] = np.eye(128) * RS2
    host["identp"] = idp.astype(bfd)
    if affine:
        host["grep"] = np.stack([
            np.broadcast_to(g[n].astype(np.float32), (128, D))
            for n in ("g1", "b1", "g2", "b2", "g3", "b3")]).copy()

    in_maps = []
    y = np.asarray(g["y"], np.float32)
    mem = np.asarray(g["mem"], np.float32)
    for c in range(N_CORES):
        sl = slice(BPC * c, BPC * (c + 1))
        m = dict(host)
        yb = y[sl]                                            # [BPC, 1024, 1024]
        m["y0b"] = np.ascontiguousarray(
            yb.reshape(BPC, 8, 128, 1024).transpose(0, 2, 1, 3)
            .reshape(BPC, 128, 8192)).astype(bfd)
        yT = yb.transpose(0, 2, 1)                            # [BPC, 1024d, 1024s]
        m["y0T8"] = np.ascontiguousarray(
            yT.reshape(BPC, 8, 128, 1024).transpose(0, 2, 1, 3)
            .reshape(BPC, 128, 8192)).astype(f8d)
        mT = mem[sl].transpose(0, 2, 1)                       # [BPC, 1024d, 2048s]
        m["memT8"] = np.ascontiguousarray(
            mT.reshape(BPC, 8, 128, 2048).transpose(0, 2, 1, 3)
            .reshape(BPC, 128, 16384)).astype(f8d)
        in_maps.append(m)
    return in_maps, affine


def kernel(**inputs):
    in_maps, affine = _prep_host(inputs)
    if affine not in _CACHE:
        _CACHE[affine] = _build(affine)
    nc = _CACHE[affine]
    res = run_bass_kernel_spmd(nc, in_maps, list(range(N_CORES)))
    return np.concatenate([r["out"] for r in res.results], axis=0)


if __name__ == "__main__":
    rng = np.random.default_rng(0)
    ins = {
        "mem": rng.standard_normal((B, S_M, D), dtype=np.float32),
        "y": rng.standard_normal((B, S_T, D), dtype=np.float32),
        **{k: (rng.standard_normal(s, dtype=np.float32) * 0.02).astype(np.float32)
           for k, s in {
               "Wq1": (H, D, DQ), "Wk1": (H, D, DQ), "Wv1": (H, D, DQ),
               "Wo1": (D, D), "Wq2": (H, D, DQ), "Wk2": (H, D, DQ),
               "Wv2": (H, D, DQ), "Wo2": (D, D), "E1": (BNK, D),
               "D1": (HID, BNK), "E2": (BNK, HID), "D2": (D, BNK)}.items()},
        "g1": np.ones(D, np.float32), "b1": np.zeros(D, np.float32),
        "g2": np.ones(D, np.float32), "b2": np.zeros(D, np.float32),
        "g3": np.ones(D, np.float32), "b3": np.zeros(D, np.float32),
    }
    o = kernel(**ins)
    print("out", o.shape, o.dtype, np.abs(o).mean())
